# revision 1
# baseline (speedup 1.0000x reference)
"""Trainium2 Bass kernel for nn_GraphSemanticExtractor (GNN message passing).

Sharding (8 NeuronCores):
  Launch A: edge build        -- core c => (batch b=c//4, row-chunk rc=c%4 of 256 rows)
  Launch B: GAT layer 1       -- core c => (batch b=c//4, head hd=c%4)
  Launch C: GAT layer 2       -- same as B, inputs are B's per-head partial outputs
  Launch D: pool + proj head  -- core c => batch b=c (2 cores)

Key idea: the sparse top-k aggregation out[dst] += wgt*h[src] is done as a dense
matmul out.T = h.T @ R with R[s,t] = ew_k(s)*exp(lrelu(e_src[s]+e_dst[t])) at
t=topi[s,k].  R is built on the vector engine with iota-compare terms
(M0 = sum_k (iota==topi_k)*ew_k) and the attention factor applied densely.
Host-side work between launches is pure gather/transpose/concat glue.
"""

import sys

sys.path.insert(0, "/opt/trn_rl_repo")
sys.path.insert(0, "/opt/trn_rl_repo/concourse")

from contextlib import ExitStack

import ml_dtypes
import numpy as np

import concourse.bass as bass
import concourse.tile as tile
from concourse import bacc, mybir
from concourse.bass_utils import run_bass_kernel_spmd

F32 = mybir.dt.float32
BF16 = mybir.dt.bfloat16
U32 = mybir.dt.uint32
AF = mybir.ActivationFunctionType
OP = mybir.AluOpType
AX = mybir.AxisListType

B, S, H = 2, 1024, 1024
HEADS, K = 4, 8
SEM = 512
NB = H // 128  # 8 partition blocks
CH = S // 4    # 256 rows per edge-build core


def _mm_loop(ctx, nc, psum_pool, lhsT, rhs, mblocks, nsize, kblocks, evict):
    """out[m,n] = sum_k lhsT[k]^T rhs[k].  lhsT(k,m)->AP [128, Mblk], rhs(k,n)->AP [128,nn].
    evict(m, n0, nn, psum_ap) stores the [128, nn] f32 psum tile."""
    for m in range(mblocks):
        n0 = 0
        while n0 < nsize:
            nn = min(512, nsize - n0)
            pt = psum_pool.tile([128, nn], F32, tag="mmp")
            for k in range(kblocks):
                nc.tensor.matmul(
                    pt[:], lhsT(k, m), rhs(k, n0, nn),
                    start=(k == 0), stop=(k == kblocks - 1),
                )
            evict(m, n0, nn, pt[:])
            n0 += nn


def _build_A(nc):
    """Edge build: inputs xT (full, transposed), xTc (row chunk), phi_w.T, psi_w.T."""
    xT = nc.dram_tensor("xT", [H, S], F32, kind="ExternalInput")
    xTc = nc.dram_tensor("xTc", [H, CH], F32, kind="ExternalInput")
    pwT = nc.dram_tensor("pwT", [H, H], F32, kind="ExternalInput")
    swT = nc.dram_tensor("swT", [H, H], F32, kind="ExternalInput")
    srcx = nc.dram_tensor("srcx", [CH, 1], F32, kind="ExternalInput")
    topi = nc.dram_tensor("topi", [CH, K], U32, kind="ExternalOutput")
    ew = nc.dram_tensor("ew", [CH, K], F32, kind="ExternalOutput")

    with tile.TileContext(nc) as tc, ExitStack() as ctx:
        pers = ctx.enter_context(tc.tile_pool(name="pers", bufs=1))
        psum = ctx.enter_context(tc.tile_pool(name="psum", bufs=6, space="PSUM"))

        xT16 = pers.tile([128, NB, S], BF16, tag="xT16")
        xTc16 = pers.tile([128, NB, CH], BF16, tag="xTc16")
        pwT16 = pers.tile([128, NB, H], BF16, tag="pwT16")
        swT16 = pers.tile([128, NB, H], BF16, tag="swT16")
        xTr = xT[:].rearrange("(kb p) s -> p kb s", p=128)
        tmpa = ctx.enter_context(tc.tile_pool(name="tmpa", bufs=3))
        for kb in range(NB):
            stg = tmpa.tile([128, S], F32, tag="stg")
            nc.sync.dma_start(out=stg[:], in_=xTr[:, kb, :])
            nc.vector.tensor_copy(out=xT16[:, kb, :], in_=stg[:])
        nc.gpsimd.dma_start(out=xTc16[:], in_=xTc[:].rearrange("(kb p) s -> p kb s", p=128))
        nc.gpsimd.dma_start(out=pwT16[:], in_=pwT[:].rearrange("(kb p) s -> p kb s", p=128))
        nc.gpsimd.dma_start(out=swT16[:], in_=swT[:].rearrange("(kb p) s -> p kb s", p=128))

        psi16 = pers.tile([128, NB, S], BF16, tag="psi16")   # psi_h.T [e, t]
        phi16 = pers.tile([128, NB, CH], BF16, tag="phi16")  # phi_h.T [e, s-chunk]

        def ev_psi(m, n0, nn, pt):
            eng = nc.scalar if (m + n0) % 2 else nc.vector
            (eng.copy if eng is nc.scalar else eng.tensor_copy)(out=psi16[:, m, n0:n0 + nn], in_=pt)

        _mm_loop(ctx, nc, psum,
                 lambda k, m: swT16[:, k, m * 128:(m + 1) * 128],
                 lambda k, n0, nn: xT16[:, k, n0:n0 + nn],
                 NB, S, NB, ev_psi)

        def ev_phi(m, n0, nn, pt):
            nc.vector.tensor_copy(out=phi16[:, m, n0:n0 + nn], in_=pt)

        _mm_loop(ctx, nc, psum,
                 lambda k, m: pwT16[:, k, m * 128:(m + 1) * 128],
                 lambda k, n0, nn: xTc16[:, k, n0:n0 + nn],
                 NB, CH, NB, ev_phi)

        # scores [s-chunk, t] f32
        sc = pers.tile([128, 2, S], F32, tag="scores")

        def ev_sc(m, n0, nn, pt):
            nc.vector.tensor_copy(out=sc[:, m, n0:n0 + nn], in_=pt)

        _mm_loop(ctx, nc, psum,
                 lambda k, m: phi16[:, k, m * 128:(m + 1) * 128],
                 lambda k, n0, nn: psi16[:, k, n0:n0 + nn],
                 2, S, NB, ev_sc)

        # top-8 per row, softmax over the 8, self-edge mask
        mv = pers.tile([128, 2, K], F32, tag="mv")
        ti = pers.tile([128, 2, K], U32, tag="ti")
        for m in range(2):
            nc.vector.max(mv[:, m, :], sc[:, m, :])
            nc.vector.max_index(ti[:, m, :], mv[:, m, :], sc[:, m, :])
        ex = pers.tile([128, 2, K], F32, tag="ex")
        nc.scalar.activation(ex[:], mv[:], AF.Exp)
        sm = pers.tile([128, 2, 1], F32, tag="sm")
        nc.vector.tensor_reduce(sm[:], ex[:], axis=AX.X, op=OP.add)
        nc.vector.tensor_scalar(sm[:], sm[:], 1e-8, None, op0=OP.add)
        rc = pers.tile([128, 2, 1], F32, tag="rc")
        nc.vector.reciprocal(rc[:], sm[:])
        sx = pers.tile([128, 2, 1], F32, tag="sx")
        nc.sync.dma_start(out=sx[:], in_=srcx[:].rearrange("(m p) c -> p m c", p=128))
        tif = pers.tile([128, 2, K], F32, tag="tif")
        nc.vector.tensor_copy(out=tif[:], in_=ti[:])
        w8 = pers.tile([128, 2, K], F32, tag="w8")
        msk = pers.tile([128, 2, K], F32, tag="msk")
        for m in range(2):
            nc.vector.tensor_scalar(w8[:, m, :], ex[:, m, :], rc[:, m, :], 1e-8, op0=OP.mult, op1=OP.max)
            nc.vector.tensor_scalar(msk[:, m, :], tif[:, m, :], sx[:, m, :], None, op0=OP.is_equal)
            nc.vector.tensor_scalar(msk[:, m, :], msk[:, m, :], -1.0, 1.0, op0=OP.mult, op1=OP.add)
        ewt = pers.tile([128, 2, K], F32, tag="ewt")
        nc.vector.tensor_tensor(ewt[:], w8[:], msk[:], op=OP.mult)
        nc.sync.dma_start(out=topi[:].rearrange("(m p) k -> p m k", p=128), in_=ti[:])
        nc.sync.dma_start(out=ew[:].rearrange("(m p) k -> p m k", p=128), in_=ewt[:])
    nc.compile()
    return nc


def _build_BC(nc, first, skip_r=False, skip_hmm=False, skip_agg=False, skip_dma=False):
    """One GAT layer for one (batch, head).  Outputs gT[feat, node] = (agg/attn)/HEADS, bf16."""
    if first:
        xT = nc.dram_tensor("xT", [H, S], F32, kind="ExternalInput")
    else:
        ps = [nc.dram_tensor(f"p{i}", [H, S], BF16, kind="ExternalInput") for i in range(4)]
    WT = nc.dram_tensor("WT", [H, H], F32, kind="ExternalInput")
    a2r = nc.dram_tensor("a2r", [2, H], F32, kind="ExternalInput")
    tpf = nc.dram_tensor("tpf", [S, K], F32, kind="ExternalInput")
    tpi = nc.dram_tensor("tpi", [S, K], mybir.dt.int16, kind="ExternalInput")
    ewd = nc.dram_tensor("ewd", [S, K], F32, kind="ExternalInput")
    iot = nc.dram_tensor("iot", [1, S], F32, kind="ExternalInput")
    gT = nc.dram_tensor("gT", [H, S], BF16, kind="ExternalOutput")

    with tile.TileContext(nc) as tc, ExitStack() as ctx:
        pers = ctx.enter_context(tc.tile_pool(name="pers", bufs=1))
        tmp = ctx.enter_context(tc.tile_pool(name="tmp", bufs=3))
        psum = ctx.enter_context(tc.tile_pool(name="psum", bufs=5, space="PSUM"))
        psmall = ctx.enter_context(tc.tile_pool(name="psmall", bufs=1, space="PSUM"))

        xT16 = pers.tile([128, NB, S], BF16, tag="xT16")
        if first:
            nc.gpsimd.dma_start(out=xT16[:], in_=xT[:].rearrange("(kb p) s -> p kb s", p=128))
        else:
            for kb in range(NB):
                pin = [tmp.tile([128, S], BF16, tag=f"pin{i}", name=f"pin{i}") for i in range(4)]
                for i in range(4):
                    nc.sync.dma_start(
                        out=pin[i][:],
                        in_=ps[i][:].rearrange("(kb p) s -> p kb s", p=128)[:, kb, :])
                a01 = tmp.tile([128, S], BF16, tag="a01")
                a23 = tmp.tile([128, S], BF16, tag="a23")
                nc.vector.tensor_tensor(a01[:], pin[0][:], pin[1][:], op=OP.add)
                nc.vector.tensor_tensor(a23[:], pin[2][:], pin[3][:], op=OP.add)
                nc.vector.tensor_tensor(a01[:], a01[:], a23[:], op=OP.add)
                nc.scalar.activation(xT16[:, kb, :], a01[:], AF.Relu)

        WT16 = pers.tile([128, NB, H], BF16, tag="WT16")
        nc.gpsimd.dma_start(out=WT16[:], in_=WT[:].rearrange("(kb p) s -> p kb s", p=128))
        a2s = pers.tile([2, H], BF16, tag="a2s")
        nc.gpsimd.dma_start(out=a2s[:], in_=a2r[:])
        asb = pers.tile([128, H], BF16, tag="asb")
        adb = pers.tile([128, H], BF16, tag="adb")
        nc.gpsimd.partition_broadcast(asb[:], a2s[0:1, :])
        a2d1 = pers.tile([1, H], BF16, tag="a2d1")
        nc.sync.dma_start(out=a2d1[:], in_=a2s[1:2, :])
        nc.gpsimd.partition_broadcast(adb[:], a2d1[:])
        tpw = pers.tile([128, NB, K], mybir.dt.int16, tag="tpw")
        nc.sync.dma_start(out=tpw[:], in_=tpi[:].rearrange("(m p) k -> p m k", p=128))
        ews16 = pers.tile([128, NB, K], BF16, tag="ews16")
        nc.gpsimd.dma_start(out=ews16[:], in_=ewd[:].rearrange("(m p) k -> p m k", p=128))

        # h [node, feat] bf16
        h16 = pers.tile([128, NB, H], BF16, tag="h16")

        def ev_h(m, n0, nn, pt):
            eng = (m + n0 // 512) % 2
            if eng:
                nc.scalar.copy(out=h16[:, m, n0:n0 + nn], in_=pt)
            else:
                nc.vector.tensor_copy(out=h16[:, m, n0:n0 + nn], in_=pt)

        if skip_hmm:
            nc.vector.memset(h16[:], 0.0)
        else:
            _mm_loop(ctx, nc, psum,
                     lambda k, m: xT16[:, k, m * 128:(m + 1) * 128],
                     lambda k, n0, nn: WT16[:, k, n0:n0 + nn],
                     NB, H, NB, ev_h)

        # V = W^T [a_src|a_dst] -> [d, 2], via row-wise reductions of WT
        Vf = pers.tile([128, NB, 2], F32, tag="Vf")
        V16 = pers.tile([128, NB, 2], BF16, tag="V16")
        for m in range(NB):
            j1 = tmp.tile([128, H], BF16, tag="j1")
            nc.vector.scalar_tensor_tensor(j1[:], WT16[:, m, :], 1.0, asb[:],
                                           op0=OP.mult, op1=OP.mult,
                                           accum_out=Vf[:, m, 0:1])
            j2 = tmp.tile([128, H], BF16, tag="j2")
            nc.vector.scalar_tensor_tensor(j2[:], WT16[:, m, :], 1.0, adb[:],
                                           op0=OP.mult, op1=OP.mult,
                                           accum_out=Vf[:, m, 1:2])
        nc.vector.tensor_copy(out=V16[:], in_=Vf[:])

        # e_bothT [2, node] = V^T x
        ebT = pers.tile([2, S], F32, tag="ebT")

        def ev_e(m, n0, nn, pt):
            nc.vector.tensor_copy(out=ebT[:, n0:n0 + nn], in_=pt)

        for n0 in range(0, S, 512):
            pt = psmall.tile([2, 512], F32, tag="ebp")
            for k in range(NB):
                nc.tensor.matmul(pt[:], V16[:, k, :], xT16[:, k, n0:n0 + 512],
                                 start=(k == 0), stop=(k == NB - 1))
            ev_e(0, n0, 512, pt[:])

        edst1 = pers.tile([1, S], F32, tag="edst1")
        nc.sync.dma_start(out=edst1[:], in_=ebT[1:2, :])
        edb = pers.tile([128, S], F32, tag="edb")
        nc.gpsimd.partition_broadcast(edb[:], edst1[:])

        ones11 = pers.tile([1, 1], F32, tag="ones11")
        nc.vector.memset(ones11[:], 1.0)
        esc = pers.tile([128, NB, 1], F32, tag="esc")
        for m in range(NB):
            pt = psmall.tile([128, 1], F32, tag="escp")
            nc.tensor.matmul(pt[:], ebT[0:1, m * 128:(m + 1) * 128], ones11[:],
                             start=True, stop=True)
            nc.vector.tensor_copy(out=esc[:, m, :], in_=pt[:])

        # R [s, t] bf16: M0 = sum_k (iota==topi_k)*ew_k, then * exp(lrelu(e_src+e_dst))
        R = pers.tile([128, NB, S], BF16, tag="R")
        for m in range(0 if skip_r else NB):
            m0 = tmp.tile([128, S], BF16, tag="m0")
            nc.gpsimd.local_scatter(m0[:], ews16[:, m, :], tpw[:, m, :],
                                    channels=128, num_elems=S, num_idxs=K)
            zl = tmp.tile([128, S], F32, tag="zl")
            nc.scalar.activation(zl[:], edb[:], AF.Lrelu, bias=esc[:, m, :], alpha=0.2)
            ez = tmp.tile([128, S], BF16, tag="ez")
            nc.scalar.activation(ez[:], zl[:], AF.Exp)
            nc.vector.tensor_tensor(R[:, m, :], m0[:], ez[:], op=OP.mult)

        # attn^T [1, t] = 1^T R ; recip = 0.25 / (attn + 1e-8)
        onesc = pers.tile([128, 1], BF16, tag="onesc")
        nc.vector.memset(onesc[:], 1.0)
        atT = pers.tile([1, S], F32, tag="atT")
        for n0 in range(0, S, 512):
            pt = psmall.tile([1, 512], F32, tag="atp")
            for k in range(NB):
                nc.tensor.matmul(pt[:], onesc[:], R[:, k, n0:n0 + 512],
                                 start=(k == 0), stop=(k == NB - 1))
            nc.vector.tensor_copy(out=atT[:, n0:n0 + 512], in_=pt[:])
        nc.vector.tensor_scalar(atT[:], atT[:], 1e-8, None, op0=OP.add)
        arc = pers.tile([1, S], F32, tag="arc")
        nc.vector.reciprocal(arc[:], atT[:])
        nc.vector.tensor_scalar(arc[:], arc[:], 1.0 / HEADS, None, op0=OP.mult)
        rcb = pers.tile([128, S], F32, tag="rcb")
        nc.gpsimd.partition_broadcast(rcb[:], arc[:])

        # out^T [feat, t] = h^T R, scaled by rcb
        gsb = pers.tile([128, NB, S], BF16, tag="gsb")

        def ev_g(m, n0, nn, pt):
            nc.vector.tensor_tensor(gsb[:, m, n0:n0 + nn], pt, rcb[:, n0:n0 + nn], op=OP.mult)

        if skip_agg:
            nc.vector.memset(gsb[:], 0.0)
        else:
            _mm_loop(ctx, nc, psum,
                     lambda k, m: h16[:, k, m * 128:(m + 1) * 128],
                     lambda k, n0, nn: R[:, k, n0:n0 + nn],
                     NB, S, NB, ev_g)
        nc.sync.dma_start(out=gT[:].rearrange("(m p) t -> p m t", p=128), in_=gsb[:])
    nc.compile()
    return nc


def _build_D(nc):
    """x3 = relu(sum of per-head partials); attention pool over nodes; 2-layer head."""
    from concourse.masks import make_identity
    ps = [nc.dram_tensor(f"p{i}", [H, S], BF16, kind="ExternalInput") for i in range(4)]
    wpc = nc.dram_tensor("wpc", [H, 1], F32, kind="ExternalInput")
    w1T = nc.dram_tensor("w1T", [H, SEM], F32, kind="ExternalInput")
    b1c = nc.dram_tensor("b1c", [SEM, 1], F32, kind="ExternalInput")
    w2T = nc.dram_tensor("w2T", [SEM, SEM], F32, kind="ExternalInput")
    b2c = nc.dram_tensor("b2c", [SEM, 1], F32, kind="ExternalInput")
    res = nc.dram_tensor("res", [SEM, 1], F32, kind="ExternalOutput")

    with tile.TileContext(nc) as tc, ExitStack() as ctx:
        pers = ctx.enter_context(tc.tile_pool(name="pers", bufs=1))
        tmp = ctx.enter_context(tc.tile_pool(name="tmp", bufs=3))
        psum = ctx.enter_context(tc.tile_pool(name="psum", bufs=6, space="PSUM"))

        x3T = pers.tile([128, NB, S], BF16, tag="x3T")
        pt_ = [pers.tile([128, NB, S], BF16, tag=f"pin{i}", name=f"pin{i}") for i in range(4)]
        for i in range(4):
            nc.sync.dma_start(out=pt_[i][:], in_=ps[i][:].rearrange("(kb p) s -> p kb s", p=128))
        for kb in range(NB):
            a01 = tmp.tile([128, S], BF16, tag="a01")
            a23 = tmp.tile([128, S], BF16, tag="a23")
            nc.vector.tensor_tensor(a01[:], pt_[0][:, kb, :], pt_[1][:, kb, :], op=OP.add)
            nc.vector.tensor_tensor(a23[:], pt_[2][:, kb, :], pt_[3][:, kb, :], op=OP.add)
            nc.vector.tensor_tensor(a01[:], a01[:], a23[:], op=OP.add)
            nc.scalar.activation(x3T[:, kb, :], a01[:], AF.Relu)

        wp16 = pers.tile([128, NB, 1], BF16, tag="wp16")
        nc.gpsimd.dma_start(out=wp16[:], in_=wpc[:].rearrange("(kb p) c -> p kb c", p=128))
        psc = pers.tile([1, S], F32, tag="psc")
        for n0 in range(0, S, 512):
            pt = psum.tile([1, 512], F32, tag="sp")
            for k in range(NB):
                nc.tensor.matmul(pt[:], wp16[:, k, :], x3T[:, k, n0:n0 + 512],
                                 start=(k == 0), stop=(k == NB - 1))
            nc.vector.tensor_copy(out=psc[:, n0:n0 + 512], in_=pt[:])

        mx = pers.tile([1, 1], F32, tag="mx")
        nc.vector.tensor_reduce(mx[:], psc[:], axis=AX.X, op=OP.max)
        nmx = pers.tile([1, 1], F32, tag="nmx")
        nc.vector.tensor_scalar(nmx[:], mx[:], -1.0, None, op0=OP.mult)
        ev = pers.tile([1, S], F32, tag="ev")
        nc.scalar.activation(ev[:], psc[:], AF.Exp, bias=nmx[:])
        sm = pers.tile([1, 1], F32, tag="sm")
        nc.vector.tensor_reduce(sm[:], ev[:], axis=AX.X, op=OP.add)
        rc = pers.tile([1, 1], F32, tag="rc")
        nc.vector.reciprocal(rc[:], sm[:])
        alT = pers.tile([1, S], BF16, tag="alT")
        nc.vector.tensor_scalar(alT[:], ev[:], rc[:], None, op0=OP.mult)

        alb = pers.tile([128, S], BF16, tag="alb")
        nc.gpsimd.partition_broadcast(alb[:], alT[:])
        pldf = pers.tile([128, NB, 1], F32, tag="pldf")
        pld = pers.tile([128, NB, 1], BF16, tag="pld")
        for m in range(NB):
            junk = tmp.tile([128, S], BF16, tag="junk")
            nc.vector.scalar_tensor_tensor(junk[:], x3T[:, m, :], 1.0, alb[:],
                                           op0=OP.mult, op1=OP.mult,
                                           accum_out=pldf[:, m, :])
        nc.vector.tensor_copy(out=pld[:], in_=pldf[:])

        w116 = pers.tile([128, NB, SEM], BF16, tag="w116")
        nc.gpsimd.dma_start(out=w116[:], in_=w1T[:].rearrange("(kb p) c -> p kb c", p=128))
        b1f = pers.tile([128, 4, 1], F32, tag="b1f")
        nc.sync.dma_start(out=b1f[:], in_=b1c[:].rearrange("(m p) c -> p m c", p=128))
        hid = pers.tile([128, 4, 1], BF16, tag="hid")
        for m in range(4):
            pt = psum.tile([128, 1], F32, tag="sp")
            for k in range(NB):
                nc.tensor.matmul(pt[:], w116[:, k, m * 128:(m + 1) * 128], pld[:, k, :],
                                 start=(k == 0), stop=(k == NB - 1))
            nc.scalar.activation(hid[:, m, :], pt[:], AF.Relu, bias=b1f[:, m, :])

        w216 = pers.tile([128, 4, SEM], BF16, tag="w216")
        nc.gpsimd.dma_start(out=w216[:], in_=w2T[:].rearrange("(kb p) c -> p kb c", p=128))
        b2f = pers.tile([128, 4, 1], F32, tag="b2f")
        nc.sync.dma_start(out=b2f[:], in_=b2c[:].rearrange("(m p) c -> p m c", p=128))
        rsb = pers.tile([128, 4, 1], F32, tag="rsb")
        for m in range(4):
            pt = psum.tile([128, 1], F32, tag="sp")
            for k in range(4):
                nc.tensor.matmul(pt[:], w216[:, k, m * 128:(m + 1) * 128], hid[:, k, :],
                                 start=(k == 0), stop=(k == 3))
            nc.vector.tensor_tensor(rsb[:, m, :], pt[:], b2f[:, m, :], op=OP.add)
        nc.sync.dma_start(out=res[:].rearrange("(m p) c -> p m c", p=128), in_=rsb[:])
    nc.compile()
    return nc


_PROGS = {}


def _get_progs():
    if not _PROGS:
        def mk():
            return bacc.Bacc("TRN2", target_bir_lowering=False, debug=False,
                             enable_asserts=True, num_devices=8)
        _PROGS["A"] = _build_A(mk())
        _PROGS["B"] = _build_BC(mk(), first=True)
        _PROGS["C"] = _build_BC(mk(), first=False)
        _PROGS["D"] = _build_D(mk())
    return _PROGS


def kernel(hidden_states, phi_w, psi_w, gat_lin_w, gat_att, wp, w1, b1, w2, b2,
           _profile=None):
    f32 = np.float32
    bf16 = ml_dtypes.bfloat16
    hidden_states = np.asarray(hidden_states, f32)
    progs = _get_progs()
    C = lambda a: np.ascontiguousarray(a)
    times = {}

    def run(tag, in_maps, core_ids):
        r = run_bass_kernel_spmd(progs[tag], in_maps, core_ids=core_ids)
        if _profile is not None:
            times[tag] = r.exec_time_ns
        return r.results

    # ---- launch A: edge build ----
    xTb = [C(hidden_states[b].T) for b in range(B)]
    pwT, swT = C(np.asarray(phi_w, f32).T), C(np.asarray(psi_w, f32).T)
    in_a = []
    for c in range(8):
        b, rcn = c // 4, c % 4
        in_a.append({
            "xT": xTb[b], "xTc": C(xTb[b][:, rcn * CH:(rcn + 1) * CH]),
            "pwT": pwT, "swT": swT,
            "srcx": C(np.arange(rcn * CH, (rcn + 1) * CH, dtype=np.float32)[:, None]),
        })
    ra = run("A", in_a, list(range(8)))
    topi = np.stack([np.concatenate([ra[b * 4 + r]["topi"] for r in range(4)], 0) for b in range(B)])
    ew = np.stack([np.concatenate([ra[b * 4 + r]["ew"] for r in range(4)], 0) for b in range(B)])
    topi_f = topi.astype(f32)
    iota = np.arange(S, dtype=f32)[None, :]

    # ---- launches B, C: the two GAT layers ----
    ga = np.asarray(gat_att, f32)
    glw = np.asarray(gat_lin_w, f32)
    prev = None
    for li, tag in enumerate(("B", "C")):
        in_l = []
        for c in range(8):
            b, hd = c // 4, c % 4
            Wm = glw[li, hd * H:(hd + 1) * H, :]
            d = {
                "WT": C(Wm.T),
                "a2r": C(ga[li, hd].reshape(2, H)),
                "tpf": C(topi_f[b]), "tpi": C(topi[b].astype(np.int16)),
                "ewd": C(ew[b]), "iot": C(iota),
            }
            if li == 0:
                d["xT"] = xTb[b]
            else:
                for i in range(4):
                    d[f"p{i}"] = prev[b * 4 + i]
            in_l.append(d)
        rl = run(tag, in_l, list(range(8)))
        prev = [np.asarray(rl[c]["gT"], bf16) for c in range(8)]

    # ---- launch D: pooling + projection head ----
    in_d = []
    for b in range(B):
        d = {f"p{i}": prev[b * 4 + i] for i in range(4)}
        d.update({
            "wpc": C(np.asarray(wp, f32).reshape(H, 1)),
            "w1T": C(np.asarray(w1, f32).T), "b1c": C(np.asarray(b1, f32)[:, None]),
            "w2T": C(np.asarray(w2, f32).T), "b2c": C(np.asarray(b2, f32)[:, None]),
        })
        in_d.append(d)
    rd = run("D", in_d, [0, 1])
    out = np.stack([rd[b]["res"][:, 0].astype(f32) for b in range(B)])
    if _profile is not None:
        _profile.update(times)
    return out



# revision 6
# speedup vs baseline: 1.3108x; 1.3108x over previous
"""Trainium2 Bass kernel for nn_GraphSemanticExtractor (GNN message passing).

Sharding (8 NeuronCores), 6 launches:
  A1: projections      -- core c => (batch b=c//4, proj pj=(c%4)//2, half hf=c%2)
                          computes phi_h/psi_h^T for 512 nodes (no redundancy)
  A2: scores + top-k   -- core c => (batch b=c//4, row-chunk rc=c%4 of 256 rows)
  B:  GAT layer 1      -- core c => (batch b=c//4, head hd=c%4)
  C:  GAT layer 2      -- same as B, inputs are B's per-head partial outputs
  D1: partial pooling  -- core c => (batch b=c//4, node-chunk q=c%4 of 256)
  D2: pool-combine+head-- core c => batch b=c (2 cores)

Key ideas vs naive:
  * scores = (x phi_w^T)(x psi_w^T)^T with the projections computed once
    across cores (A1) instead of per-core.
  * sparse top-k aggregation out^T = h^T R with R[s,t] = ew_k(s) *
    exp(lrelu(e_src[s]+e_dst[t])) at t=topi[s,k] done as dense matmul; the
    attention factor uses exp(lrelu(x)) == max(exp(x), exp(0.2x)), which
    factorizes over s and t -- no dense lrelu/exp passes, no activation
    table thrash; R = max(a1[s]b1[t], a2[s]b2[t]) * scatter(ew).
  * k-outer matmul accumulation so the PE starts while input DMAs stream.
  * attention pooling split: per-chunk exp-weighted partial sums (D1),
    globally combined on 2 cores (D2).
"""

import sys

sys.path.insert(0, "/opt/trn_rl_repo")
sys.path.insert(0, "/opt/trn_rl_repo/concourse")

from contextlib import ExitStack

import ml_dtypes
import numpy as np

import concourse.bass as bass
import concourse.tile as tile
from concourse import bacc, mybir
from concourse.bass_utils import run_bass_kernel_spmd

F32 = mybir.dt.float32
BF16 = mybir.dt.bfloat16
U32 = mybir.dt.uint32
I16 = mybir.dt.int16
AF = mybir.ActivationFunctionType
OP = mybir.AluOpType
AX = mybir.AxisListType

B, S, H = 2, 1024, 1024
HEADS, K = 4, 8
SEM = 512
NB = H // 128   # 8 partition blocks
CH = S // 4     # 256 rows per A2/D1 core
HF = S // 2     # 512 cols per A1 core


def _r(dram, p=128):
    """[ (kb p) x ] dram -> [p, kb, x] AP."""
    return dram[:].rearrange("(kb p) x -> p kb x", p=p)


def _build_A1(nc):
    """One projection (phi or psi) for one column-half of one batch.
    pT[e, n] = sum_d w^T[d, e] x^T[d, n]   (contraction over feature d)."""
    wT = nc.dram_tensor("wT", [H, H], BF16, kind="ExternalInput")
    xTh = nc.dram_tensor("xTh", [H, HF], BF16, kind="ExternalInput")
    pT = nc.dram_tensor("pT", [H, HF], BF16, kind="ExternalOutput")

    with tile.TileContext(nc) as tc, ExitStack() as ctx:
        pers = ctx.enter_context(tc.tile_pool(name="pers", bufs=1))
        psum = ctx.enter_context(tc.tile_pool(name="psum", bufs=1, space="PSUM"))

        w16 = pers.tile([128, NB, H], BF16, tag="w16")
        x16 = pers.tile([128, NB, HF], BF16, tag="x16")
        o16 = pers.tile([128, NB, HF], BF16, tag="o16")
        wr, xr = _r(wT), _r(xTh)
        for j in range(4):
            sl = slice(2 * j, 2 * j + 2)
            nc.sync.dma_start(out=w16[:, sl, :], in_=wr[:, sl, :])
            nc.sync.dma_start(out=x16[:, sl, :], in_=xr[:, sl, :])

        oR = _r(pT)
        for g in range(2):
            ms = range(4 * g, 4 * g + 4)
            pts = [psum.tile([128, HF], F32, tag=f"pt{m % 4}", name=f"pt{m % 4}") for m in ms]
            for k in range(NB):
                for i, m in enumerate(ms):
                    nc.tensor.matmul(pts[i][:], w16[:, k, m * 128:(m + 1) * 128],
                                     x16[:, k, :], start=(k == 0), stop=(k == NB - 1))
            for i, m in enumerate(ms):
                if i % 2:
                    nc.scalar.copy(out=o16[:, m, :], in_=pts[i][:])
                else:
                    nc.vector.tensor_copy(out=o16[:, m, :], in_=pts[i][:])
            nc.sync.dma_start(out=oR[:, 4 * g:4 * g + 4, :], in_=o16[:, 4 * g:4 * g + 4, :])
    nc.compile()
    return nc


def _build_A2(nc):
    """scores[s, t] = phi_h[s] . psi_h[t] for a 256-row chunk; top-8 + edge w."""
    ps0 = nc.dram_tensor("ps0", [H, HF], BF16, kind="ExternalInput")
    ps1 = nc.dram_tensor("ps1", [H, HF], BF16, kind="ExternalInput")
    phc = nc.dram_tensor("phc", [H, CH], BF16, kind="ExternalInput")
    srcx = nc.dram_tensor("srcx", [CH, 1], F32, kind="ExternalInput")
    topi = nc.dram_tensor("topi", [CH, K], U32, kind="ExternalOutput")
    ew = nc.dram_tensor("ew", [CH, K], F32, kind="ExternalOutput")

    with tile.TileContext(nc) as tc, ExitStack() as ctx:
        pers = ctx.enter_context(tc.tile_pool(name="pers", bufs=1))
        psum = ctx.enter_context(tc.tile_pool(name="psum", bufs=6, space="PSUM"))

        ps16 = pers.tile([128, NB, S], BF16, tag="ps16")
        ph16 = pers.tile([128, NB, CH], BF16, tag="ph16")
        nc.sync.dma_start(out=ph16[:], in_=_r(phc))
        nc.sync.dma_start(out=ps16[:, :, 0:HF], in_=_r(ps0))
        nc.sync.dma_start(out=ps16[:, :, HF:S], in_=_r(ps1))

        sc = pers.tile([128, 2, S], F32, tag="sc")
        for m in range(2):
            for n0 in range(0, S, 512):
                pt = psum.tile([128, 512], F32, tag="pt")
                for k in range(NB):
                    nc.tensor.matmul(pt[:], ph16[:, k, m * 128:(m + 1) * 128],
                                     ps16[:, k, n0:n0 + 512],
                                     start=(k == 0), stop=(k == NB - 1))
                eng = nc.scalar if (m + n0 // 512) % 2 else nc.vector
                (eng.copy if eng is nc.scalar else eng.tensor_copy)(
                    out=sc[:, m, n0:n0 + 512], in_=pt[:])

        # top-8 per row, softmax over the 8, self-edge mask
        mv = pers.tile([128, 2, K], F32, tag="mv")
        ti = pers.tile([128, 2, K], U32, tag="ti")
        for m in range(2):
            nc.vector.max(mv[:, m, :], sc[:, m, :])
            nc.vector.max_index(ti[:, m, :], mv[:, m, :], sc[:, m, :])
        ex = pers.tile([128, 2, K], F32, tag="ex")
        nc.scalar.activation(ex[:], mv[:], AF.Exp)
        sm = pers.tile([128, 2, 1], F32, tag="sm")
        nc.vector.tensor_reduce(sm[:], ex[:], axis=AX.X, op=OP.add)
        nc.vector.tensor_scalar(sm[:], sm[:], 1e-8, None, op0=OP.add)
        rc = pers.tile([128, 2, 1], F32, tag="rc")
        nc.vector.reciprocal(rc[:], sm[:])
        sx = pers.tile([128, 2, 1], F32, tag="sx")
        nc.sync.dma_start(out=sx[:], in_=srcx[:].rearrange("(m p) c -> p m c", p=128))
        tif = pers.tile([128, 2, K], F32, tag="tif")
        nc.vector.tensor_copy(out=tif[:], in_=ti[:])
        w8 = pers.tile([128, 2, K], F32, tag="w8")
        msk = pers.tile([128, 2, K], F32, tag="msk")
        for m in range(2):
            nc.vector.tensor_scalar(w8[:, m, :], ex[:, m, :], rc[:, m, :], 1e-8,
                                    op0=OP.mult, op1=OP.max)
            nc.vector.tensor_scalar(msk[:, m, :], tif[:, m, :], sx[:, m, :], None,
                                    op0=OP.is_equal)
            nc.vector.tensor_scalar(msk[:, m, :], msk[:, m, :], -1.0, 1.0,
                                    op0=OP.mult, op1=OP.add)
        ewt = pers.tile([128, 2, K], F32, tag="ewt")
        nc.vector.tensor_tensor(ewt[:], w8[:], msk[:], op=OP.mult)
        nc.sync.dma_start(out=topi[:].rearrange("(m p) k -> p m k", p=128), in_=ti[:])
        nc.sync.dma_start(out=ew[:].rearrange("(m p) k -> p m k", p=128), in_=ewt[:])
    nc.compile()
    return nc


def _build_BC(nc, first):
    """One GAT layer for one (batch, head). gT[feat, node] = (agg/attn)/HEADS."""
    if first:
        xT = nc.dram_tensor("xT", [H, S], BF16, kind="ExternalInput")
    else:
        ps = [nc.dram_tensor(f"p{i}", [H, S], BF16, kind="ExternalInput") for i in range(4)]
    WT = nc.dram_tensor("WT", [H, H], BF16, kind="ExternalInput")
    a2r = nc.dram_tensor("a2r", [2, H], F32, kind="ExternalInput")
    tpi = nc.dram_tensor("tpi", [S, K], I16, kind="ExternalInput")
    ewd = nc.dram_tensor("ewd", [S, K], F32, kind="ExternalInput")
    gT = nc.dram_tensor("gT", [H, S], BF16, kind="ExternalOutput")

    with tile.TileContext(nc) as tc, ExitStack() as ctx:
        pers = ctx.enter_context(tc.tile_pool(name="pers", bufs=1))
        tadd = ctx.enter_context(tc.tile_pool(name="tadd", bufs=2))
        tv = ctx.enter_context(tc.tile_pool(name="tv", bufs=2))
        tr = ctx.enter_context(tc.tile_pool(name="tr", bufs=2))
        psum = ctx.enter_context(tc.tile_pool(name="psum", bufs=1, space="PSUM"))
        psmall = ctx.enter_context(tc.tile_pool(name="psmall", bufs=1, space="PSUM"))

        xT16 = pers.tile([128, NB, S], BF16, tag="xT16")
        WT16 = pers.tile([128, NB, H], BF16, tag="WT16")
        WTr = _r(WT)
        if first:
            xTr = _r(xT)
            for j in range(4):
                sl = slice(2 * j, 2 * j + 2)
                nc.sync.dma_start(out=WT16[:, sl, :], in_=WTr[:, sl, :])
                nc.sync.dma_start(out=xT16[:, sl, :], in_=xTr[:, sl, :])
        else:
            nc.sync.dma_start(out=WT16[:, 0:4, :], in_=WTr[:, 0:4, :])
            nc.sync.dma_start(out=WT16[:, 4:8, :], in_=WTr[:, 4:8, :])
            prs = [_r(p) for p in ps]
            for j in range(4):
                sl = slice(2 * j, 2 * j + 2)
                pin = [tadd.tile([128, 2, S], BF16, tag=f"pin{i}", name=f"pin{i}")
                       for i in range(4)]
                for i in range(4):
                    nc.sync.dma_start(out=pin[i][:], in_=prs[i][:, sl, :])
                a01 = tadd.tile([128, 2, S], BF16, tag="a01")
                a23 = tadd.tile([128, 2, S], BF16, tag="a23")
                nc.vector.tensor_tensor(a01[:], pin[0][:], pin[1][:], op=OP.add)
                nc.vector.tensor_tensor(a23[:], pin[2][:], pin[3][:], op=OP.add)
                nc.vector.tensor_tensor(a01[:], a01[:], a23[:], op=OP.add)
                nc.scalar.activation(xT16[:, sl, :], a01[:], AF.Relu)

        # small inputs on the SWDGE queue
        a2s = pers.tile([2, H], BF16, tag="a2s")
        nc.gpsimd.dma_start(out=a2s[:], in_=a2r[:])
        tpw = pers.tile([128, NB, K], I16, tag="tpw")
        nc.gpsimd.dma_start(out=tpw[:], in_=tpi[:].rearrange("(m p) k -> p m k", p=128))
        ews16 = pers.tile([128, NB, K], BF16, tag="ews16")
        nc.gpsimd.dma_start(out=ews16[:], in_=ewd[:].rearrange("(m p) k -> p m k", p=128))

        # V = W^T [a_src|a_dst] -> [d, 2] via row-wise reductions of WT
        asb = pers.tile([128, H], BF16, tag="asb")
        adb = pers.tile([128, H], BF16, tag="adb")
        nc.gpsimd.partition_broadcast(asb[:], a2s[0:1, :])
        a2d1 = pers.tile([1, H], BF16, tag="a2d1")
        nc.sync.dma_start(out=a2d1[:], in_=a2s[1:2, :])
        nc.gpsimd.partition_broadcast(adb[:], a2d1[:])
        Vf = pers.tile([128, NB, 2], F32, tag="Vf")
        V16 = pers.tile([128, NB, 2], BF16, tag="V16")
        for m in range(NB):
            j1 = tv.tile([128, H], BF16, tag="j1")
            nc.vector.scalar_tensor_tensor(j1[:], WT16[:, m, :], 1.0, asb[:],
                                           op0=OP.mult, op1=OP.mult,
                                           accum_out=Vf[:, m, 0:1])
            j2 = tv.tile([128, H], BF16, tag="j2")
            nc.vector.scalar_tensor_tensor(j2[:], WT16[:, m, :], 1.0, adb[:],
                                           op0=OP.mult, op1=OP.mult,
                                           accum_out=Vf[:, m, 1:2])
        nc.vector.tensor_copy(out=V16[:], in_=Vf[:])

        # e_bothT [2, node] = V^T x
        ebT = pers.tile([2, S], F32, tag="ebT")
        for n0 in range(0, S, 512):
            pt = psmall.tile([2, 512], F32, tag="ebp")
            for k in range(NB):
                nc.tensor.matmul(pt[:], V16[:, k, :], xT16[:, k, n0:n0 + 512],
                                 start=(k == 0), stop=(k == NB - 1))
            nc.vector.tensor_copy(out=ebT[:, n0:n0 + 512], in_=pt[:])

        # e_src into partition layout via transpose-matmul trick
        ones11 = pers.tile([1, 1], F32, tag="ones11")
        nc.vector.memset(ones11[:], 1.0)
        esc = pers.tile([128, NB, 1], F32, tag="esc")
        for m in range(NB):
            pt = psmall.tile([128, 1], F32, tag="escp")
            nc.tensor.matmul(pt[:], ebT[0:1, m * 128:(m + 1) * 128], ones11[:],
                             start=True, stop=True)
            nc.vector.tensor_copy(out=esc[:, m, :], in_=pt[:])

        # factored attention: exp(lrelu(es+ed)) = max(e^es e^ed, e^.2es e^.2ed)
        a1 = pers.tile([128, NB, 1], F32, tag="a1")
        a2f = pers.tile([128, NB, 1], F32, tag="a2f")
        nc.scalar.activation(a1[:], esc[:], AF.Exp)
        nc.scalar.activation(a2f[:], esc[:], AF.Exp, scale=0.2)
        e1 = pers.tile([1, S], F32, tag="e1")
        nc.sync.dma_start(out=e1[:], in_=ebT[1:2, :])
        b1 = pers.tile([1, S], BF16, tag="b1")
        b2 = pers.tile([1, S], BF16, tag="b2")
        nc.scalar.activation(b1[:], e1[:], AF.Exp)
        nc.scalar.activation(b2[:], e1[:], AF.Exp, scale=0.2)
        b1b = pers.tile([128, S], BF16, tag="b1b")
        b2b = pers.tile([128, S], BF16, tag="b2b")
        nc.gpsimd.partition_broadcast(b1b[:], b1[:])
        nc.gpsimd.partition_broadcast(b2b[:], b2[:])

        # h [node, feat] bf16, k-outer so PE starts while DMAs stream
        h16 = pers.tile([128, NB, H], BF16, tag="h16")
        for n0 in range(0, H, 512):
            for g in range(2):
                ms = range(4 * g, 4 * g + 4)
                pts = [psum.tile([128, 512], F32, tag=f"hp{m % 4}", name=f"hp{m % 4}")
                       for m in ms]
                for k in range(NB):
                    for i, m in enumerate(ms):
                        nc.tensor.matmul(pts[i][:], xT16[:, k, m * 128:(m + 1) * 128],
                                         WT16[:, k, n0:n0 + 512],
                                         start=(k == 0), stop=(k == NB - 1))
                for i, m in enumerate(ms):
                    if i % 2:
                        nc.scalar.copy(out=h16[:, m, n0:n0 + 512], in_=pts[i][:])
                    else:
                        nc.vector.tensor_copy(out=h16[:, m, n0:n0 + 512], in_=pts[i][:])

        # R [s, t] bf16 = scatter(ew) * max(a1[s]b1[t], a2[s]b2[t])
        R = pers.tile([128, NB, S], BF16, tag="R")
        for m in range(NB):
            m0 = tr.tile([128, S], BF16, tag="m0")
            nc.gpsimd.local_scatter(m0[:], ews16[:, m, :], tpw[:, m, :],
                                    channels=128, num_elems=S, num_idxs=K)
            t2 = tr.tile([128, S], BF16, tag="t2")
            nc.vector.tensor_scalar(t2[:], b2b[:], a2f[:, m, :], None, op0=OP.mult)
            u = tr.tile([128, S], BF16, tag="u")
            nc.vector.scalar_tensor_tensor(u[:], b1b[:], a1[:, m, :], t2[:],
                                           op0=OP.mult, op1=OP.max)
            nc.vector.tensor_tensor(R[:, m, :], u[:], m0[:], op=OP.mult)

        # attn^T [1, t] = 1^T R ; rcb = (1/HEADS) / (attn + 1e-8)
        onesc = pers.tile([128, 1], BF16, tag="onesc")
        nc.vector.memset(onesc[:], 1.0)
        atT = pers.tile([1, S], F32, tag="atT")
        for n0 in range(0, S, 512):
            pt = psmall.tile([1, 512], F32, tag="atp")
            for k in range(NB):
                nc.tensor.matmul(pt[:], onesc[:], R[:, k, n0:n0 + 512],
                                 start=(k == 0), stop=(k == NB - 1))
            nc.vector.tensor_copy(out=atT[:, n0:n0 + 512], in_=pt[:])
        nc.vector.tensor_scalar(atT[:], atT[:], 1e-8, None, op0=OP.add)
        arc = pers.tile([1, S], F32, tag="arc")
        nc.vector.reciprocal(arc[:], atT[:])
        nc.vector.tensor_scalar(arc[:], arc[:], 1.0 / HEADS, None, op0=OP.mult)
        rcb = pers.tile([128, S], F32, tag="rcb")
        nc.gpsimd.partition_broadcast(rcb[:], arc[:])

        # out^T [feat, t] = h^T R, scaled by rcb at eviction
        gsb = pers.tile([128, NB, S], BF16, tag="gsb")
        gTr = _r(gT)
        for m in range(NB):
            for n0 in range(0, S, 512):
                pt = psum.tile([128, 512], F32, tag=f"hp{(2 * m + n0 // 512) % 4}",
                               name="gp")
                for k in range(NB):
                    nc.tensor.matmul(pt[:], h16[:, k, m * 128:(m + 1) * 128],
                                     R[:, k, n0:n0 + 512],
                                     start=(k == 0), stop=(k == NB - 1))
                nc.vector.tensor_tensor(gsb[:, m, n0:n0 + 512], pt[:],
                                        rcb[:, n0:n0 + 512], op=OP.mult)
            if m % 2:
                nc.sync.dma_start(out=gTr[:, m - 1:m + 1, :], in_=gsb[:, m - 1:m + 1, :])
    nc.compile()
    return nc


def _build_D1(nc):
    """x3 = relu(sum heads) for a 256-node chunk; exp(score)-weighted partials."""
    ps = [nc.dram_tensor(f"p{i}", [H, CH], BF16, kind="ExternalInput") for i in range(4)]
    wpc = nc.dram_tensor("wpc", [H, 1], F32, kind="ExternalInput")
    Pp = nc.dram_tensor("Pp", [H, 1], F32, kind="ExternalOutput")
    S1 = nc.dram_tensor("S1", [1, 1], F32, kind="ExternalOutput")

    with tile.TileContext(nc) as tc, ExitStack() as ctx:
        pers = ctx.enter_context(tc.tile_pool(name="pers", bufs=1))
        tmp = ctx.enter_context(tc.tile_pool(name="tmp", bufs=4))
        psum = ctx.enter_context(tc.tile_pool(name="psum", bufs=4, space="PSUM"))

        x3T = pers.tile([128, NB, CH], BF16, tag="x3T")
        pt_ = [pers.tile([128, NB, CH], BF16, tag=f"pin{i}", name=f"pin{i}")
               for i in range(4)]
        for i in range(4):
            nc.sync.dma_start(out=pt_[i][:], in_=_r(ps[i]))
        wp16 = pers.tile([128, NB, 1], BF16, tag="wp16")
        nc.gpsimd.dma_start(out=wp16[:], in_=_r(wpc))
        for kb in range(NB):
            a01 = tmp.tile([128, CH], BF16, tag="a01")
            a23 = tmp.tile([128, CH], BF16, tag="a23")
            nc.vector.tensor_tensor(a01[:], pt_[0][:, kb, :], pt_[1][:, kb, :], op=OP.add)
            nc.vector.tensor_tensor(a23[:], pt_[2][:, kb, :], pt_[3][:, kb, :], op=OP.add)
            nc.vector.tensor_tensor(a01[:], a01[:], a23[:], op=OP.add)
            nc.scalar.activation(x3T[:, kb, :], a01[:], AF.Relu)

        # scores for this chunk, then z = exp(score) (|score| << 1, safe)
        pt = psum.tile([1, CH], F32, tag="sp")
        for k in range(NB):
            nc.tensor.matmul(pt[:], wp16[:, k, :], x3T[:, k, :],
                             start=(k == 0), stop=(k == NB - 1))
        z = pers.tile([1, CH], F32, tag="z")
        nc.scalar.activation(z[:], pt[:], AF.Exp)
        s1t = pers.tile([1, 1], F32, tag="s1t")
        nc.vector.tensor_reduce(s1t[:], z[:], axis=AX.X, op=OP.add)
        z16 = pers.tile([1, CH], BF16, tag="z16")
        nc.vector.tensor_copy(out=z16[:], in_=z[:])
        zb = pers.tile([128, CH], BF16, tag="zb")
        nc.gpsimd.partition_broadcast(zb[:], z16[:])

        # P[d] = sum_s z[s] x3[d, s]
        Pf = pers.tile([128, NB, 1], F32, tag="Pf")
        for kb in range(NB):
            junk = tmp.tile([128, CH], BF16, tag="junk")
            nc.vector.scalar_tensor_tensor(junk[:], x3T[:, kb, :], 1.0, zb[:],
                                           op0=OP.mult, op1=OP.mult,
                                           accum_out=Pf[:, kb, :])
        nc.sync.dma_start(out=Pp[:].rearrange("(kb p) c -> p kb c", p=128), in_=Pf[:])
        nc.sync.dma_start(out=S1[:], in_=s1t[:])
    nc.compile()
    return nc


def _build_D2(nc):
    """Combine pooling partials; 2-layer projection head."""
    Ps = [nc.dram_tensor(f"P{i}", [H, 1], F32, kind="ExternalInput") for i in range(4)]
    S1s = nc.dram_tensor("S1s", [1, 4], F32, kind="ExternalInput")
    w1T = nc.dram_tensor("w1T", [H, SEM], BF16, kind="ExternalInput")
    b1c = nc.dram_tensor("b1c", [SEM, 1], F32, kind="ExternalInput")
    w2T = nc.dram_tensor("w2T", [SEM, SEM], BF16, kind="ExternalInput")
    b2c = nc.dram_tensor("b2c", [SEM, 1], F32, kind="ExternalInput")
    res = nc.dram_tensor("res", [SEM, 1], F32, kind="ExternalOutput")

    with tile.TileContext(nc) as tc, ExitStack() as ctx:
        pers = ctx.enter_context(tc.tile_pool(name="pers", bufs=1))
        psum = ctx.enter_context(tc.tile_pool(name="psum", bufs=4, space="PSUM"))

        w116 = pers.tile([128, NB, SEM], BF16, tag="w116")
        nc.sync.dma_start(out=w116[:], in_=_r(w1T))
        w216 = pers.tile([128, 4, SEM], BF16, tag="w216")
        nc.sync.dma_start(out=w216[:], in_=_r(w2T))
        Pts = [pers.tile([128, NB, 1], F32, tag=f"Pt{i}", name=f"Pt{i}")
               for i in range(4)]
        for i in range(4):
            nc.gpsimd.dma_start(out=Pts[i][:], in_=_r(Ps[i]))
        s14 = pers.tile([1, 4], F32, tag="s14")
        nc.gpsimd.dma_start(out=s14[:], in_=S1s[:])
        b1f = pers.tile([128, 4, 1], F32, tag="b1f")
        nc.gpsimd.dma_start(out=b1f[:], in_=b1c[:].rearrange("(m p) c -> p m c", p=128))
        b2f = pers.tile([128, 4, 1], F32, tag="b2f")
        nc.gpsimd.dma_start(out=b2f[:], in_=b2c[:].rearrange("(m p) c -> p m c", p=128))

        Psum = pers.tile([128, NB, 1], F32, tag="Psum")
        nc.vector.tensor_tensor(Psum[:], Pts[0][:], Pts[1][:], op=OP.add)
        Psb = pers.tile([128, NB, 1], F32, tag="Psb")
        nc.vector.tensor_tensor(Psb[:], Pts[2][:], Pts[3][:], op=OP.add)
        nc.vector.tensor_tensor(Psum[:], Psum[:], Psb[:], op=OP.add)
        s1 = pers.tile([1, 1], F32, tag="s1")
        nc.vector.tensor_reduce(s1[:], s14[:], axis=AX.X, op=OP.add)
        rc1 = pers.tile([1, 1], F32, tag="rc1")
        nc.vector.reciprocal(rc1[:], s1[:])
        rcb = pers.tile([128, 1], F32, tag="rcb")
        nc.gpsimd.partition_broadcast(rcb[:], rc1[:])
        pld = pers.tile([128, NB, 1], BF16, tag="pld")
        nc.vector.tensor_scalar(pld[:], Psum[:], rcb[:, 0:1], None, op0=OP.mult)

        hid = pers.tile([128, 4, 1], BF16, tag="hid")
        for m in range(4):
            pt = psum.tile([128, 1], F32, tag="sp")
            for k in range(NB):
                nc.tensor.matmul(pt[:], w116[:, k, m * 128:(m + 1) * 128], pld[:, k, :],
                                 start=(k == 0), stop=(k == NB - 1))
            nc.scalar.activation(hid[:, m, :], pt[:], AF.Relu, bias=b1f[:, m, :])

        rsb = pers.tile([128, 4, 1], F32, tag="rsb")
        for m in range(4):
            pt = psum.tile([128, 1], F32, tag="sp")
            for k in range(4):
                nc.tensor.matmul(pt[:], w216[:, k, m * 128:(m + 1) * 128], hid[:, k, :],
                                 start=(k == 0), stop=(k == 3))
            nc.vector.tensor_tensor(rsb[:, m, :], pt[:], b2f[:, m, :], op=OP.add)
        nc.sync.dma_start(out=res[:].rearrange("(m p) c -> p m c", p=128), in_=rsb[:])
    nc.compile()
    return nc


_PROGS = {}


def _get_progs():
    if not _PROGS:
        def mk():
            return bacc.Bacc("TRN2", target_bir_lowering=False, debug=False,
                             enable_asserts=True, num_devices=8)
        _PROGS["A1"] = _build_A1(mk())
        _PROGS["A2"] = _build_A2(mk())
        _PROGS["B"] = _build_BC(mk(), first=True)
        _PROGS["C"] = _build_BC(mk(), first=False)
        _PROGS["D1"] = _build_D1(mk())
        _PROGS["D2"] = _build_D2(mk())
    return _PROGS


def kernel(hidden_states, phi_w, psi_w, gat_lin_w, gat_att, wp, w1, b1, w2, b2,
           _profile=None):
    f32 = np.float32
    bf16 = ml_dtypes.bfloat16
    hidden_states = np.asarray(hidden_states, f32)
    progs = _get_progs()
    C = lambda a: np.ascontiguousarray(a)
    times = {}

    def run(tag, in_maps, core_ids):
        r = run_bass_kernel_spmd(progs[tag], in_maps, core_ids=core_ids)
        if _profile is not None:
            times[tag] = r.exec_time_ns
        return r.results

    # ---- A1: projections (phi_h / psi_h transposed, bf16) ----
    xTb = [C(hidden_states[b].T.astype(bf16)) for b in range(B)]
    pwT = C(np.asarray(phi_w, f32).T.astype(bf16))
    swT = C(np.asarray(psi_w, f32).T.astype(bf16))
    in_a1 = []
    for c in range(8):
        b, pj, hf = c // 4, (c % 4) // 2, c % 2
        in_a1.append({
            "wT": pwT if pj == 0 else swT,
            "xTh": C(xTb[b][:, hf * HF:(hf + 1) * HF]),
        })
    ra1 = run("A1", in_a1, list(range(8)))
    phiT = [[ra1[b * 4 + hf]["pT"] for hf in range(2)] for b in range(B)]
    psiT = [[ra1[b * 4 + 2 + hf]["pT"] for hf in range(2)] for b in range(B)]

    # ---- A2: scores chunk + top-8 + edge weights ----
    in_a2 = []
    for c in range(8):
        b, rcn = c // 4, c % 4
        hf, qr = rcn // 2, rcn % 2
        in_a2.append({
            "ps0": psiT[b][0], "ps1": psiT[b][1],
            "phc": C(np.asarray(phiT[b][hf])[:, qr * CH:(qr + 1) * CH]),
            "srcx": C(np.arange(rcn * CH, (rcn + 1) * CH, dtype=f32)[:, None]),
        })
    ra2 = run("A2", in_a2, list(range(8)))
    topi = np.stack([np.concatenate([ra2[b * 4 + r]["topi"] for r in range(4)], 0)
                     for b in range(B)])
    ew = np.stack([np.concatenate([ra2[b * 4 + r]["ew"] for r in range(4)], 0)
                   for b in range(B)])

    # ---- B, C: the two GAT layers ----
    ga = np.asarray(gat_att, f32)
    glw = np.asarray(gat_lin_w, f32)
    prev = None
    for li, tag in enumerate(("B", "C")):
        in_l = []
        for c in range(8):
            b, hd = c // 4, c % 4
            Wm = glw[li, hd * H:(hd + 1) * H, :]
            d = {
                "WT": C(Wm.T.astype(bf16)),
                "a2r": C(ga[li, hd].reshape(2, H)),
                "tpi": C(topi[b].astype(np.int16)),
                "ewd": C(ew[b]),
            }
            if li == 0:
                d["xT"] = xTb[b]
            else:
                for i in range(4):
                    d[f"p{i}"] = prev[b * 4 + i]
            in_l.append(d)
        rl = run(tag, in_l, list(range(8)))
        prev = [np.asarray(rl[c]["gT"], bf16) for c in range(8)]

    # ---- D1: per-chunk pooling partials ----
    in_d1 = []
    for c in range(8):
        b, q = c // 4, c % 4
        d = {f"p{i}": C(np.asarray(prev[b * 4 + i])[:, q * CH:(q + 1) * CH])
             for i in range(4)}
        d["wpc"] = C(np.asarray(wp, f32).reshape(H, 1))
        in_d1.append(d)
    rd1 = run("D1", in_d1, list(range(8)))

    # ---- D2: combine + projection head ----
    in_d2 = []
    for b in range(B):
        d = {f"P{i}": rd1[b * 4 + i]["Pp"] for i in range(4)}
        d["S1s"] = C(np.concatenate([rd1[b * 4 + i]["S1"] for i in range(4)], 1))
        d.update({
            "w1T": C(np.asarray(w1, f32).T.astype(bf16)), "b1c": C(np.asarray(b1, f32)[:, None]),
            "w2T": C(np.asarray(w2, f32).T.astype(bf16)), "b2c": C(np.asarray(b2, f32)[:, None]),
        })
        in_d2.append(d)
    rd2 = run("D2", in_d2, [0, 1])
    out = np.stack([rd2[b]["res"][:, 0].astype(f32) for b in range(B)])
    if _profile is not None:
        _profile.update(times)
    return out
